# revision 9
# baseline (speedup 1.0000x reference)
"""Trainium2 Bass kernel for nn_Attention_83502754169400.

Tensor-parallel over heads across 8 NeuronCores:
  - each core projects q/k/v for its 2 heads (fp16 matmuls, fp32 PSUM accum),
  - qk-LayerNorm + RoPE (rotate-half form via host-permuted weight columns),
  - causal attention with transposed scores (no max-subtraction; scores are
    O(1) after LN so exp never overflows), softmax Z via an appended
    ones-column on V,
  - AllToAll exchanges per-head attention outputs so each core owns 512
    full output rows,
  - row-block matmul against the full wo.
Host does layout prep (transpose of x, weight slicing/permutation, fp16
casts) and concatenates the per-core row blocks.
"""

import math

import numpy as np

B, S, D, H = 2, 2048, 2048, 16
HD = D // H          # 128
NCORES = 8
HL = H // NCORES     # heads per core = 2
DL = HL * HD         # per-core projected width = 256
EPS = 1e-5
SCALE = 1.0 / math.sqrt(HD)

NTB = (B * S) // 128          # 32 token blocks of 128
NQB = S // 128                # 16 blocks per batch
NDB = D // 128                # 16 contraction blocks
TOK_CHUNK = 256               # xT DMA chunk (tokens)

_RUNNER = None


# ----------------------------------------------------------------- device ---

def _build_nc():
    import concourse.bass as bass  # noqa: F401
    import concourse.mybir as mybir
    import concourse.tile as tile
    from concourse import bacc

    f16 = mybir.dt.float16
    f32 = mybir.dt.float32
    AF = mybir.ActivationFunctionType

    nc = bacc.Bacc("TRN2", target_bir_lowering=False, debug=False,
                   enable_asserts=False, num_devices=NCORES)

    xt = nc.dram_tensor("xt", [D, B * S], f16, kind="ExternalInput")
    wqkv = nc.dram_tensor("wqkv", [128, NDB, 3 * DL], f16, kind="ExternalInput")
    wo = nc.dram_tensor("wo", [D, D], f16, kind="ExternalInput")
    abcd_q = nc.dram_tensor("abcd_q", [128, NQB, 4, 64], f16, kind="ExternalInput")
    abcd_k = nc.dram_tensor("abcd_k", [128, NQB, 4, 64], f16, kind="ExternalInput")
    maskt = nc.dram_tensor("maskt", [128, 128], f32, kind="ExternalInput")
    ident = nc.dram_tensor("ident", [128, 128], f16, kind="ExternalInput")
    out = nc.dram_tensor("out", [B * S // NCORES, D], f32, kind="ExternalOutput")

    with tile.TileContext(nc) as tc:
        with (
            tc.tile_pool(name="persist", bufs=1) as persist,
            tc.tile_pool(name="xtp", bufs=2) as xtp,
            tc.tile_pool(name="qn", bufs=4) as qnp,
            tc.tile_pool(name="qr", bufs=4) as qrp,
            tc.tile_pool(name="rt", bufs=2) as rtp,
            tc.tile_pool(name="stat", bufs=4) as statp,
            tc.tile_pool(name="exps", bufs=18) as expp,
            tc.tile_pool(name="wop", bufs=18) as wop,
            tc.tile_pool(name="osb", bufs=3) as osbp,
            tc.tile_pool(name="otsb", bufs=4) as otsbp,
            tc.tile_pool(name="rz", bufs=4) as rzp,
            tc.tile_pool(name="proj", bufs=2, space="PSUM") as projp,
            tc.tile_pool(name="stps", bufs=2, space="PSUM") as stp,
            tc.tile_pool(name="acc", bufs=2, space="PSUM") as accp,
            tc.tile_pool(name="dram", bufs=1, space="DRAM") as dram,
        ):
            # ---- persistent tensors
            wqkv_sb = persist.tile([128, NDB, 3 * DL], f16)
            qt_sb = persist.tile([128, HL, NTB, 128], f16)
            kt_sb = persist.tile([128, HL, NTB, 128], f16)
            vaug_sb = persist.tile([128, NTB, HL, 132], f16)
            abq_sb = persist.tile([128, NQB, 4, 64], f16)
            abk_sb = persist.tile([128, NQB, 4, 64], f16)
            maskt_sb = persist.tile([128, 128], f32)
            ident_sb = persist.tile([128, 128], f16)
            attnt_sb = persist.tile([128, NDB, 512], f16)
            eps_sb = persist.tile([128, 1], f32)
            nc.vector.memset(eps_sb[:], EPS)

            nc.sync.dma_start(wqkv_sb[:], wqkv.ap())
            nc.sync.dma_start(abq_sb[:], abcd_q.ap())
            nc.sync.dma_start(abk_sb[:], abcd_k.ap())
            nc.sync.dma_start(maskt_sb[:], maskt.ap())
            nc.sync.dma_start(ident_sb[:], ident.ap())
            nc.vector.memset(vaug_sb[:, :, :, 128:129], 1.0)

            # ---- projections + LN + RoPE + transpose, per 128-token block
            n_chunks = (B * S) // TOK_CHUNK
            tpc = TOK_CHUNK // 128  # token blocks per chunk
            for ch in range(n_chunks):
                xt_sb = xtp.tile([128, NDB, TOK_CHUNK], f16,
                                 name=f"xt{ch}", tag="xt")
                for db in range(NDB):
                    nc.sync.dma_start(
                        xt_sb[:, db, :],
                        xt.ap()[db * 128:(db + 1) * 128,
                                ch * TOK_CHUNK:(ch + 1) * TOK_CHUNK])
                for tl in range(tpc):
                    tb = ch * tpc + tl
                    proj = projp.tile([128, 6, 128], f32,
                                      name=f"proj{tb}", tag="proj")
                    for db in range(NDB):
                        lhs = xt_sb[:, db, tl * 128:(tl + 1) * 128]
                        nc.tensor.matmul(proj[:, 0:4, :], lhs,
                                         wqkv_sb[:, db, 0:512],
                                         start=(db == 0), stop=(db == NDB - 1))
                        nc.tensor.matmul(proj[:, 4:6, :], lhs,
                                         wqkv_sb[:, db, 512:768],
                                         start=(db == 0), stop=(db == NDB - 1))

                    qb = tb % NQB
                    for (nm, off, ab_sb, tdst) in (("q", 0, abq_sb, qt_sb),
                                                   ("k", 2, abk_sb, kt_sb)):
                        stats = statp.tile([128, 2, 6], f32,
                                           name=f"sa_{nm}{tb}", tag="lnstats")
                        aggr = statp.tile([128, 2, 2], f32,
                                          name=f"ag_{nm}{tb}", tag="lnaggr")
                        std = statp.tile([128, 2], f32,
                                         name=f"sd_{nm}{tb}", tag="lnstd")
                        rstd = statp.tile([128, 2], f32,
                                          name=f"rs_{nm}{tb}", tag="lnrstd")
                        nmr = statp.tile([128, 2], f32,
                                         name=f"nm_{nm}{tb}", tag="lnnmr")
                        qn = qnp.tile([128, 2, 128], f16,
                                      name=f"qn_{nm}{tb}", tag="qn")
                        qr = qrp.tile([128, 2, 128], f16,
                                      name=f"qr_{nm}{tb}", tag="qr")
                        for h in range(HL):
                            nc.vector.bn_stats(stats[:, h, :],
                                               proj[:, off + h, :])
                            nc.vector.bn_aggr(aggr[:, h, :], stats[:, h, :])
                            nc.scalar.activation(std[:, h:h + 1], aggr[:, h, 1:2],
                                                 AF.Sqrt, bias=eps_sb[:])
                            nc.vector.reciprocal(rstd[:, h:h + 1], std[:, h:h + 1])
                            nc.vector.scalar_tensor_tensor(
                                nmr[:, h:h + 1], aggr[:, h, 0:1], -1.0,
                                rstd[:, h:h + 1],
                                op0=mybir.AluOpType.mult, op1=mybir.AluOpType.mult)
                            nc.scalar.activation(qn[:, h, :], proj[:, off + h, :],
                                                 AF.Identity,
                                                 bias=nmr[:, h:h + 1],
                                                 scale=rstd[:, h:h + 1])
                        # rotate-half RoPE on both heads at once
                        e = qn[:, :, 0:64]
                        o = qn[:, :, 64:128]
                        A = ab_sb[:, qb, 0, :].unsqueeze(1).broadcast_to([128, 2, 64])
                        Bc = ab_sb[:, qb, 1, :].unsqueeze(1).broadcast_to([128, 2, 64])
                        C = ab_sb[:, qb, 2, :].unsqueeze(1).broadcast_to([128, 2, 64])
                        Dc = ab_sb[:, qb, 3, :].unsqueeze(1).broadcast_to([128, 2, 64])
                        t1 = rtp.tile([128, 2, 64], f16,
                                      name=f"t1_{nm}{tb}", tag="t1")
                        t2 = rtp.tile([128, 2, 64], f16,
                                      name=f"t2_{nm}{tb}", tag="t2")
                        nc.vector.tensor_mul(t1[:], e, A)
                        nc.vector.tensor_mul(t2[:], o, Bc)
                        nc.vector.tensor_sub(qr[:, :, 0:64], t1[:], t2[:])
                        nc.vector.tensor_mul(t1[:], e, C)
                        nc.vector.tensor_mul(t2[:], o, Dc)
                        nc.vector.tensor_add(qr[:, :, 64:128], t1[:], t2[:])
                        for h in range(HL):
                            tps = stp.tile([128, 128], f16,
                                           name=f"tp_{nm}{tb}{h}", tag="st")
                            nc.tensor.transpose(tps[:], qr[:, h, :], ident_sb[:])
                            nc.vector.tensor_copy(tdst[:, h, tb, :], tps[:])
                    for h in range(HL):
                        nc.scalar.copy(vaug_sb[:, tb, h, 0:128], proj[:, 4 + h, :])

            # ---- attention (transposed scores, causal), per (batch, head)
            cin = dram.tile([NCORES, DL, 512], f16)
            cout = dram.tile([NCORES, DL, 512], f16)
            for b in range(B):
                for h in range(HL):
                    for qc in range(NQB // 4):      # 512-query chunks
                        exps = []
                        for kj in range(4 * qc + 4):
                            vs = max(kj - 4 * qc, 0) * 128
                            st = stp.tile([128, 512], f32,
                                          name=f"s{b}{h}{qc}_{kj}", tag="st")
                            nc.tensor.matmul(
                                st[:, vs:512],
                                kt_sb[:, h, b * NQB + kj, :],
                                qt_sb[:, h, b * NQB + qc * 4 + vs // 128:
                                      b * NQB + qc * 4 + 4, :],
                                start=True, stop=True)
                            if kj >= 4 * qc:
                                nc.vector.tensor_add(st[:, vs:vs + 128],
                                                     st[:, vs:vs + 128],
                                                     maskt_sb[:])
                            pe = expp.tile([128, 512], f16,
                                           name=f"p{b}{h}{qc}_{kj}", tag="pe")
                            nc.scalar.activation(pe[:, vs:512], st[:, vs:512],
                                                 AF.Exp, scale=SCALE)
                            exps.append(pe)
                        for ql in range(4):
                            qb = qc * 4 + ql
                            oacc = accp.tile([128, 512], f32,
                                             name=f"o{b}{h}{qb}", tag="acc")
                            for kj in range(qb + 1):
                                nc.tensor.matmul(
                                    oacc[:, 0:129],
                                    exps[kj][:, ql * 128:(ql + 1) * 128],
                                    vaug_sb[:, b * NQB + kj, h, 0:129],
                                    start=(kj == 0), stop=(kj == qb))
                            rz = rzp.tile([128, 1], f32,
                                          name=f"rz{b}{h}{qb}", tag="rz")
                            nc.vector.reciprocal(rz[:], oacc[:, 128:129])
                            asb = qnp.tile([128, 128], f16,
                                           name=f"a{b}{h}{qb}", tag="asb")
                            nc.scalar.activation(asb[:], oacc[:, 0:128],
                                                 AF.Copy, scale=rz[:])
                            ot = stp.tile([128, 128], f16,
                                          name=f"ot{b}{h}{qb}", tag="st")
                            nc.tensor.transpose(ot[:], asb[:], ident_sb[:])
                            osb = otsbp.tile([128, 128], f16,
                                             name=f"ob{b}{h}{qb}", tag="otsb")
                            nc.vector.tensor_copy(osb[:], ot[:])
                            flat = (b * NQB + qb) * 128
                            nc.sync.dma_start(
                                cin[flat // 512,
                                    h * 128:(h + 1) * 128,
                                    (flat % 512):(flat % 512) + 128],
                                osb[:])

            nc.gpsimd.collective_compute(
                "AllToAll", mybir.AluOpType.bypass,
                replica_groups=[list(range(NCORES))],
                ins=[cin[:]], outs=[cout[:]])

            # ---- output projection: rows owned by this core
            for db in range(NDB):
                nc.sync.dma_start(
                    attnt_sb[:, db, :],
                    cout[db // 2, (db % 2) * 128:(db % 2) * 128 + 128, :])
            for nch in range(4):
                wo_t = []
                for db in range(NDB):
                    wt = wop.tile([128, 512], f16, name=f"wo{nch}_{db}", tag="wot")
                    nc.sync.dma_start(
                        wt[:], wo.ap()[db * 128:(db + 1) * 128,
                                       nch * 512:(nch + 1) * 512])
                    wo_t.append(wt)
                for tl in range(4):
                    ops = accp.tile([128, 512], f32, name=f"op{nch}_{tl}", tag="acc")
                    for db in range(NDB):
                        nc.tensor.matmul(ops[:],
                                         attnt_sb[:, db, tl * 128:(tl + 1) * 128],
                                         wo_t[db][:],
                                         start=(db == 0), stop=(db == NDB - 1))
                    osb2 = osbp.tile([128, 512], f32, name=f"os{nch}_{tl}", tag="osb")
                    nc.scalar.copy(osb2[:], ops[:])
                    nc.sync.dma_start(
                        out.ap()[tl * 128:(tl + 1) * 128,
                                 nch * 512:(nch + 1) * 512], osb2[:])

    nc.compile()
    return nc


# ------------------------------------------------------------------- host ---

def _prep_in_maps(inputs):
    f16 = np.float16
    x = np.ascontiguousarray(inputs["x"].astype(np.float32).reshape(B * S, D))
    fc = inputs["freqs_cos"].astype(np.float32)
    fs = inputs["freqs_sin"].astype(np.float32)
    mask = inputs["mask"].astype(np.float32)
    qw = inputs["q_ln_w"].astype(np.float32)
    kw = inputs["k_ln_w"].astype(np.float32)

    perm = np.concatenate([np.arange(0, 128, 2), np.arange(1, 128, 2)])
    xt16 = np.ascontiguousarray(x.T).astype(f16)

    def abcd_host(w):
        wp = w[perm]
        # [S, 4, 64] -> device layout [128 part(pos%128), 16 qb, 4, 64]
        a = np.stack([fc * wp[:64], fs * wp[64:], fs * wp[:64], fc * wp[64:]],
                     axis=1).astype(f16)               # [S, 4, 64]
        return np.ascontiguousarray(
            a.reshape(NQB, 128, 4, 64).transpose(1, 0, 2, 3))

    abcd_q = abcd_host(qw)
    abcd_k = abcd_host(kw)
    maskt = np.ascontiguousarray(np.clip(mask[:128, :128].T, -1e4, None)
                                 ).astype(np.float32)
    ident = np.eye(128, dtype=f16)
    wo16 = inputs["wo"].astype(f16)

    def wqkv_host(c):
        cols = []
        for w, permute in ((inputs["wq"], True), (inputs["wk"], True),
                           (inputs["wv"], False)):
            wc = w[:, c * DL:(c + 1) * DL].astype(np.float32).copy()
            if permute:
                for h in range(HL):
                    wc[:, h * 128:(h + 1) * 128] = wc[:, h * 128 + perm]
            cols.append(wc)
        wall = np.concatenate(cols, axis=1).astype(f16)          # [D, 768]
        # device layout [128 part, 16 dblk, 768]
        return np.ascontiguousarray(
            wall.reshape(NDB, 128, 3 * DL).transpose(1, 0, 2))

    in_maps = []
    for c in range(NCORES):
        in_maps.append({
            "xt": xt16, "wqkv": wqkv_host(c), "wo": wo16,
            "abcd_q": abcd_q, "abcd_k": abcd_k,
            "maskt": maskt, "ident": ident,
        })
    return in_maps


def _check_supported(inputs):
    if inputs["x"].shape != (B, S, D):
        return False
    mask = np.asarray(inputs["mask"], dtype=np.float32)
    if mask.shape != (S, S):
        return False
    tri = np.tril(np.ones((256, 256), dtype=bool))
    m = mask[:256, :256]
    if not (np.all(m[tri] == 0.0) and np.all(m[~tri] < -1e8)):
        return False
    if np.any(mask[S - 1, :] != 0.0) or np.any(mask[1:, 0] != 0.0):
        return False
    if np.any(inputs["q_ln_b"]) or np.any(inputs["k_ln_b"]):
        return False  # roped-beta path not built; fall back
    return True


def _get_runner():
    global _RUNNER
    if _RUNNER is None:
        _RUNNER = _build_nc()
    return _RUNNER


def _kernel_bass(inputs):
    from concourse import bass_utils
    nc = _get_runner()
    in_maps = _prep_in_maps(inputs)
    res = bass_utils.run_bass_kernel_spmd(nc, in_maps,
                                          core_ids=list(range(NCORES)))
    rows = np.concatenate([res.results[c]["out"] for c in range(NCORES)], axis=0)
    return np.ascontiguousarray(rows.reshape(B, S, D).astype(np.float32))


# --------------------------------------------------------------- fallback ---

def _kernel_jax(inputs):
    import jax
    import jax.numpy as jnp

    devs = jax.devices()[:NCORES]
    x = inputs["x"].astype(np.float32)
    fc = inputs["freqs_cos"].astype(np.float32)
    fs = inputs["freqs_sin"].astype(np.float32)
    mask = inputs["mask"].astype(np.float32)
    wq, wk, wv, wo = (inputs[k].astype(np.float32)
                      for k in ("wq", "wk", "wv", "wo"))
    qw, qb = inputs["q_ln_w"].astype(np.float32), inputs["q_ln_b"].astype(np.float32)
    kw, kb = inputs["k_ln_w"].astype(np.float32), inputs["k_ln_b"].astype(np.float32)

    wq_s = np.stack([wq[:, c * DL:(c + 1) * DL] for c in range(NCORES)])
    wk_s = np.stack([wk[:, c * DL:(c + 1) * DL] for c in range(NCORES)])
    wv_s = np.stack([wv[:, c * DL:(c + 1) * DL] for c in range(NCORES)])
    wo_s = np.stack([wo[c * DL:(c + 1) * DL, :] for c in range(NCORES)])

    def _ln(t, w, b):
        mu = jnp.mean(t, axis=-1, keepdims=True)
        var = jnp.mean(jnp.square(t - mu), axis=-1, keepdims=True)
        return (t - mu) * jax.lax.rsqrt(var + EPS) * w + b

    def _rope(t, c, s):
        e, o = t[..., 0::2], t[..., 1::2]
        cc = c[None, :, None, :]
        ss = s[None, :, None, :]
        oe = e * cc - o * ss
        oo = e * ss + o * cc
        return jnp.stack([oe, oo], axis=-1).reshape(t.shape)

    def shard_fn(wq_c, wk_c, wv_c, wo_c, x_c, fc_c, fs_c, m_c, qw_c, qb_c, kw_c, kb_c):
        b, s, _ = x_c.shape
        q = (x_c.reshape(b * s, D) @ wq_c).reshape(b, s, HL, HD)
        k = (x_c.reshape(b * s, D) @ wk_c).reshape(b, s, HL, HD)
        v = (x_c.reshape(b * s, D) @ wv_c).reshape(b, s, HL, HD)
        q = _rope(_ln(q, qw_c, qb_c), fc_c, fs_c)
        k = _rope(_ln(k, kw_c, kb_c), fc_c, fs_c)
        scores = jnp.einsum("bqhd,bkhd->bhqk", q, k) * SCALE
        scores = scores + m_c[None, None, :, :]
        probs = jax.nn.softmax(scores, axis=-1)
        o = jnp.einsum("bhqk,bkhd->bqhd", probs, v).reshape(b, s, HL * HD)
        part = o.reshape(b * s, HL * HD) @ wo_c
        return jax.lax.psum(part.reshape(b, s, D), "i")

    pfn = jax.pmap(shard_fn, axis_name="i",
                   in_axes=(0, 0, 0, 0, None, None, None, None,
                            None, None, None, None),
                   devices=devs)
    res = pfn(wq_s, wk_s, wv_s, wo_s, x, fc, fs, mask, qw, qb, kw, kb)
    return np.asarray(res[0], dtype=np.float32)


def _kernel_numpy(inputs):
    x = inputs["x"].astype(np.float32)
    fc, fs = inputs["freqs_cos"], inputs["freqs_sin"]
    mask = inputs["mask"]
    wq, wk, wv, wo = inputs["wq"], inputs["wk"], inputs["wv"], inputs["wo"]
    qw, qb = inputs["q_ln_w"], inputs["q_ln_b"]
    kw, kb = inputs["k_ln_w"], inputs["k_ln_b"]

    def ln(t, w, b):
        mu = t.mean(-1, keepdims=True)
        var = ((t - mu) ** 2).mean(-1, keepdims=True)
        return (t - mu) / np.sqrt(var + EPS) * w + b

    def rope(t):
        e, o = t[..., 0::2], t[..., 1::2]
        c = fc[None, :, None, :]
        s = fs[None, :, None, :]
        o2 = np.empty_like(t)
        o2[..., 0::2] = e * c - o * s
        o2[..., 1::2] = e * s + o * c
        return o2

    b, s, _ = x.shape
    q = rope(ln((x @ wq).reshape(b, s, H, HD), qw, qb))
    k = rope(ln((x @ wk).reshape(b, s, H, HD), kw, kb))
    v = (x @ wv).reshape(b, s, H, HD)
    o = np.empty((b, s, H, HD), dtype=np.float32)
    for bi in range(b):
        for h in range(H):
            sc = (q[bi, :, h, :] @ k[bi, :, h, :].T) * SCALE + mask
            sc -= sc.max(-1, keepdims=True)
            p = np.exp(sc)
            p /= p.sum(-1, keepdims=True)
            o[bi, :, h, :] = p @ v[bi, :, h, :]
    return (o.reshape(b, s, D) @ wo).astype(np.float32)


def kernel(**inputs) -> np.ndarray:
    try:
        if _check_supported(inputs):
            return _kernel_bass(inputs)
    except Exception:
        import traceback
        traceback.print_exc()
    try:
        return _kernel_jax(inputs)
    except Exception:
        return _kernel_numpy(inputs)
